# revision 14
# baseline (speedup 1.0000x reference)
"""DGN (3-layer GCN + max/mean pooling) Trainium2 kernel, 8-core SPMD.

Layout strategy:
- Nodes re-indexed into a padded, uniform layout: each graph padded to GP
  nodes (GP = 128*ceil(max_graph_size/128)); G/8 graphs per core, so every
  core owns the same number of nodes at identical block positions (SPMD:
  one instruction stream, per-core tensor data).
- Message passing: agg[v] = sum_{e: dst=v} table[src_e] where
  table = dis * (x @ W) (src-side deg scale folded into the table, dst-side
  folded into the epilogue relu's per-partition activation scale).
- Gathers: dma_gather (int16 idx) from the compact bf16 table viewed as
  256B pair-rows; 2 ranges x 2 parities make all nodes int16-addressable.
- Aggregation: per 128-slot gather tile one matmul with a streamed
  [128, 32] 0/1 lhsT into a 32-aligned PSUM window (slots bucketed per
  (chunk, group, 32-node window), padded to 128*k uniformly across cores).
- Inter-layer exchange: AllGather of per-core bf16 h-shards into the table.
- Pooling: per-block masked ones-matmul into a PSUM gsum accumulator and a
  per-block running max; graphs never straddle cores or blocks.
"""

import numpy as np
import ml_dtypes

import concourse.bass as bass
import concourse.bacc as bacc
import concourse.mybir as mybir
import concourse.tile as tile
from concourse.bass_utils import run_bass_kernel_spmd
from concourse.vector_clock import ScopedClock

# ---------------------------------------------------------------- tile patch
# This walrus build rejects >2 sync waits on one TPB_CTRL instruction; split
# the TileContext exit drain's waits across nop carriers.


def _split_drain_and_barrier(self, tick_clock, wait_clock):
    nc = self.nc
    drain_inst = nc.sync.drain()
    wait_clock.add_sem_waits(
        drain_inst.ins, ScopedClock({None: tick_clock.global_clock})
    )
    waits = list(drain_inst.ins.sync_info.on_wait or [])
    if len(waits) > 1:
        bb = nc.cur_bb.bb
        assert bb.instructions[-1] is drain_inst.ins, "drain not last in bb"
        bb.instructions.pop()
        drain_inst.ins.sync_info.on_wait = [waits[-1]]
        for w in waits[:-1]:
            carrier = nc.sync.nop(nofuse=True, hint="drain_wait_split")
            if carrier.ins.sync_info is None:
                carrier.ins.sync_info = mybir.SyncInfo(on_wait=[w], on_update=[])
            else:
                carrier.ins.sync_info.on_wait = [w]
        bb.instructions.append(drain_inst.ins)

    nc.all_engine_barrier()
    assert self.sems is not None
    popped = nc._tile_sem_poison_stack.pop()
    assert popped is self._sem_poison
    nc.clear_and_free_semaphores(list(self.sems.allocated().values()))
    nc.all_engine_barrier()


tile.TileContext._drain_and_barrier = _split_drain_and_barrier

f32 = mybir.dt.float32
bf16 = mybir.dt.bfloat16
i16 = mybir.dt.int16

NC = 8          # cores
D = 64          # feature dim
L = 3           # layers
WIN = 32        # psum window (nodes per lhsT)
BLK = 128       # nodes per block
NGRP = 4        # gather groups: 2 ranges x 2 parities
MAXI16 = 32767  # pair rows per range (int16 limit)


class Meta:
    pass


# ================================================================ host prep
def _preprocess(x, edge_index, batch, Ws, bs):
    m = Meta()
    N = x.shape[0]
    G = int(batch.max()) + 1
    assert G % NC == 0, G
    src = np.asarray(edge_index[0], np.int64)
    dst = np.asarray(edge_index[1], np.int64)
    loop = np.arange(N, dtype=np.int64)
    src = np.concatenate([src, loop])
    dst = np.concatenate([dst, loop])

    deg = np.bincount(dst, minlength=N).astype(np.float64)
    dis = (1.0 / np.sqrt(np.maximum(deg, 1.0))).astype(np.float32)
    dis[deg == 0] = 0.0

    counts = np.bincount(batch, minlength=G)
    assert counts.min() > 0
    GP = BLK * int(np.ceil(counts.max() / BLK))
    g_per_core = G // NC
    core_nodes = g_per_core * GP
    NT = NC * core_nodes
    CH = BLK
    for cand in (1024, 512, 256, 128):
        if core_nodes % cand == 0:
            CH = cand
            break
    nchunks = core_nodes // CH
    BPC = CH // BLK
    nblocks = core_nodes // BLK
    blocks_per_graph = GP // BLK

    gstart = np.zeros(G + 1, np.int64)
    gstart[1:] = np.cumsum(counts)
    pid = (batch * GP + (np.arange(N) - gstart[batch])).astype(np.int64)

    npair = NT // 2
    NRANGE = int(np.ceil(npair / MAXI16))
    assert NRANGE <= 2, (NT, NRANGE)
    rng_rows = int(np.ceil(npair / NRANGE))

    src_p = pid[src]
    dst_p = pid[dst]
    pair = src_p >> 1
    par = (src_p & 1).astype(np.int64)
    rngi = pair // rng_rows
    idx16 = (pair - rngi * rng_rows).astype(np.int64)
    grp = 2 * rngi + par

    core = dst_p // core_nodes
    winid = (dst_p % core_nodes) // WIN
    wincol = dst_p % WIN
    nwin = core_nodes // WIN
    wins_per_chunk = CH // WIN

    key = ((core * NGRP + grp) * nwin + winid)
    nkeys = NC * NGRP * nwin
    cnt = np.bincount(key, minlength=nkeys).reshape(NC, NGRP, nwin)
    cnt_max = cnt.max(axis=0)                              # [NGRP, nwin]
    tiles = np.ceil(cnt_max / 128.0).astype(np.int64)      # [NGRP, nwin]
    slots_per = tiles * 128

    order = np.argsort(key, kind="stable")
    key_s = key[order]
    idx16_s = idx16[order]
    wincol_s = wincol[order]
    bounds = np.searchsorted(key_s, np.arange(nkeys + 1))

    # instruction per (chunk, grp); idx stream = concat over chunk's windows
    Sgrp = np.zeros((nchunks, NGRP), np.int64)
    for k in range(nchunks):
        for g in range(NGRP):
            Sgrp[k, g] = slots_per[g, wins_per_chunk * k:wins_per_chunk * (k + 1)].sum()
    gi_off = np.zeros((nchunks, NGRP), np.int64)
    off = 0
    for k in range(nchunks):
        for g in range(NGRP):
            gi_off[k, g] = off
            off += Sgrp[k, g]
    total_slots = off
    IW = total_slots // 16

    idx_streams = np.zeros((NC, total_slots), np.int16)
    lob = np.full((NC, total_slots), -1, np.int8)  # lhsT col per slot (-1 pad)
    for c in range(NC):
        for k in range(nchunks):
            for g in range(NGRP):
                pos = gi_off[k, g]
                for w in range(wins_per_chunk * k, wins_per_chunk * (k + 1)):
                    kk = (c * NGRP + g) * nwin + w
                    lo, hi = bounds[kk], bounds[kk + 1]
                    n = hi - lo
                    sp = slots_per[g, w]
                    assert n <= sp
                    idx_streams[c, pos:pos + n] = idx16_s[lo:hi]
                    lob[c, pos:pos + n] = wincol_s[lo:hi]
                    pos += sp

    # jobs: one per 128-slot tile (uniform across cores)
    jobs = []  # (chunk, grp, tile_in_instr, block_in_chunk, win_base, start, stop)
    for k in range(nchunks):
        for g in range(NGRP):
            t = 0
            for w in range(wins_per_chunk * k, wins_per_chunk * (k + 1)):
                wl = w - wins_per_chunk * k
                for _ in range(tiles[g, w]):
                    jobs.append([k, g, t, wl // (BLK // WIN), (wl % (BLK // WIN)) * WIN,
                                 False, False])
                    t += 1
    njobs = len(jobs)

    lhs = np.zeros((NC, 128, 32 * njobs), ml_dtypes.bfloat16)
    for j, (k, g, t, jb, wb, _st, _sp) in enumerate(jobs):
        s0 = gi_off[k, g] + t * 128
        for c in range(NC):
            cols = lob[c, s0:s0 + 128].astype(np.int64)
            rows = np.nonzero(cols >= 0)[0]
            lhs[c, rows, 32 * j + cols[rows]] = 1.0

    idx_wrapped = np.zeros((NC, 128, max(IW, 1)), np.int16)
    for c in range(NC):
        for k in range(nchunks):
            for g in range(NGRP):
                S = int(Sgrp[k, g])
                if S == 0:
                    continue
                o = int(gi_off[k, g])
                wr = idx_streams[c, o:o + S].reshape(S // 16, 16).T
                idx_wrapped[c, :, o // 16:(o + S) // 16] = np.tile(wr, (8, 1))

    valid = np.zeros(NT, bool)
    valid[pid] = True
    dis_pad = np.zeros(NT, np.float32)
    dis_pad[pid] = dis
    disb = np.zeros((NC, 128, nblocks), np.float32)
    member = np.zeros((NC, 128, nblocks, g_per_core), np.float32)
    for c in range(NC):
        ids = c * core_nodes + np.arange(core_nodes)
        disb[c] = dis_pad[ids].reshape(nblocks, BLK).T
        vb = valid[ids].reshape(nblocks, BLK).T
        for b in range(nblocks):
            member[c, :, b, b // blocks_per_graph] = vb[:, b]
    disall = dis_pad.reshape(NT // BLK, BLK).T.astype(np.float32)  # [128, NT/128]

    xpad = np.zeros((NT, D), np.float32)
    xpad[pid] = np.asarray(x, np.float32)
    m.xT = np.ascontiguousarray(xpad.T).astype(ml_dtypes.bfloat16)  # [64, NT]

    m.N, m.G, m.GP, m.NT = N, G, GP, NT
    m.CH, m.nchunks, m.nblocks, m.BPC = CH, nchunks, nblocks, BPC
    m.core_nodes, m.g_per_core, m.blocks_per_graph = core_nodes, g_per_core, blocks_per_graph
    m.rng_rows = rng_rows
    m.Sgrp, m.gi_off, m.IW, m.jobs, m.njobs = Sgrp, gi_off, max(IW, 1), jobs, njobs
    m.idx_wrapped, m.lhs = idx_wrapped, lhs
    m.disb, m.member, m.disall = disb, member, disall
    m.pid, m.counts = pid, counts
    m.Ws = [np.asarray(W, np.float32) for W in Ws]
    m.bs = [np.asarray(b, np.float32) for b in bs]
    assert all(np.abs(b).max() == 0 for b in m.bs), "nonzero bias unsupported"
    m.tab_rows = NT // 2 + 8
    return m


# ============================================================= kernel build
def _build_kernel(m, repeat=1, debug=()):
    nc = bacc.Bacc(None, target_bir_lowering=False, debug=False)
    CH, nchunks, nblocks, BPC = m.CH, m.nchunks, m.nblocks, m.BPC
    gpc = m.g_per_core
    NJ = m.njobs
    NTB = m.NT // BLK

    xT_ext = nc.declare_dram_parameter("xT", [D, m.NT], bf16, isOutput=False)
    idx_ext = nc.declare_dram_parameter("idx", [128, m.IW], i16, isOutput=False)
    lhs_ext = nc.declare_dram_parameter("lhs", [128, 32 * NJ], bf16, isOutput=False)
    disb_ext = nc.declare_dram_parameter("disb", [128, nblocks], f32, isOutput=False)
    disa_ext = nc.declare_dram_parameter("disall", [128, NTB], f32, isOutput=False)
    mem_ext = nc.declare_dram_parameter("member", [128, nblocks, gpc], f32, isOutput=False)
    w_ext = nc.declare_dram_parameter("w", [D, L, D], bf16, isOutput=False)
    ident_ext = nc.declare_dram_parameter("ident", [128, 128], f32, isOutput=False)

    xout_ext = nc.declare_dram_parameter("xout", [m.core_nodes, D], f32, isOutput=True)
    pmax_ext = nc.declare_dram_parameter("pmax", [D, L, gpc], f32, isOutput=True)
    psm_ext = nc.declare_dram_parameter("psumout", [32, L, D], f32, isOutput=True)

    with tile.TileContext(nc) as tc:
        with (
            tc.tile_pool(name="const", bufs=1) as constp,
            tc.tile_pool(name="state", bufs=1) as statep,
            tc.tile_pool(name="gat", bufs=3) as gatp,
            tc.tile_pool(name="lhsp", bufs=3) as lhsp,
            tc.tile_pool(name="xtp", bufs=3) as xtp,
            tc.tile_pool(name="hps", bufs=3) as hpsp,
            tc.tile_pool(name="ps_agg", bufs=2, space="PSUM") as ps_agg,
            tc.tile_pool(name="ps_t", bufs=2, space="PSUM") as ps_t,
            tc.tile_pool(name="ps_h", bufs=2, space="PSUM") as ps_h,
            tc.tile_pool(name="ps_pool", bufs=1, space="PSUM") as ps_pool,
            tc.tile_pool(name="dram", bufs=1, space="DRAM") as dramp,
        ):
            idx_t = constp.tile([128, m.IW], i16)
            nc.sync.dma_start(out=idx_t[:], in_=idx_ext[:])
            disb_t = constp.tile([128, nblocks], f32)
            nc.sync.dma_start(out=disb_t[:], in_=disb_ext[:])
            disa_t = constp.tile([128, NTB], f32)
            nc.sync.dma_start(out=disa_t[:], in_=disa_ext[:])
            mem_t = constp.tile([128, nblocks, gpc], f32)
            nc.sync.dma_start(out=mem_t[:], in_=mem_ext[:])
            w_t = constp.tile([D, L, D], bf16)
            nc.sync.dma_start(out=w_t[:], in_=w_ext[:])
            ident_t = constp.tile([128, 128], f32)
            nc.sync.dma_start(out=ident_t[:], in_=ident_ext[:])
            zb = constp.tile([128, 512], bf16)
            nc.vector.memset(zb[:], 0.0)

            x_cur = statep.tile([128, nblocks, D], f32)
            gmax = statep.tile([128, gpc, D], f32)
            pool_sb = statep.tile([D, L, gpc], f32)
            psum_sb = statep.tile([32, L, D], f32)
            nc.vector.memset(psum_sb[:], 0.0)
            hstage = statep.tile([128, nblocks, D], bf16)

            tables = [
                dramp.tile([m.tab_rows, 128], bf16, name="tab0"),
                dramp.tile([m.tab_rows, 128], bf16, addr_space="Shared", name="tab1"),
                dramp.tile([m.tab_rows, 128], bf16, addr_space="Shared", name="tab2"),
            ]
            ag_ins = [None,
                      dramp.tile([m.core_nodes, D], bf16, name="agin1"),
                      dramp.tile([m.core_nodes, D], bf16, name="agin2")]

            def table_view(layer, r, p):
                tb = tables[layer][:, :]
                off = tb.offset + 128 * m.rng_rows * r + 64 * p
                return bass.AP(tb.tensor, off, [[128, m.rng_rows], [1, 128]])

            def a_phase(layer, full):
                tb = tables[layer][:, :]
                tabt, tab_off = tb.tensor, tb.offset
                nck = (m.NT // CH) if full else nchunks
                for k in range(nck):
                    psH = ps_h.tile([128, BPC, D], f32, tag="psH")
                    xt_c = xtp.tile([D, BPC, 128], bf16, tag="xt")
                    if full:
                        nc.sync.dma_start(
                            out=xt_c[:], in_=xT_ext[:, k * CH:(k + 1) * CH]
                        )
                    else:
                        for b in range(BPC):
                            psT = ps_t.tile([D, 128], f32, tag="psT")
                            nc.tensor.transpose(
                                out=psT[:], in_=x_cur[:, k * BPC + b, :],
                                identity=ident_t[:],
                            )
                            nc.scalar.activation(
                                out=xt_c[:, b, :], in_=psT[:],
                                func=mybir.ActivationFunctionType.Copy,
                            )
                    for b in range(BPC):
                        nc.tensor.matmul(
                            out=psH[:, b, :], lhsT=xt_c[:, b, :],
                            rhs=w_t[:, layer, :], start=True, stop=True,
                        )
                    dv = disa_t if full else disb_t
                    if full:
                        hp = hpsp.tile([128, BPC, D], bf16, tag="hp")
                        nc.vector.tensor_tensor(
                            out=hp[:], in0=psH[:],
                            in1=dv[:, k * BPC:(k + 1) * BPC, None]
                            .to_broadcast([128, BPC, D]),
                            op=mybir.AluOpType.mult,
                        )
                        dst = bass.AP(
                            tabt, tab_off + D * CH * k,
                            [[D, 128], [BLK * D, BPC], [1, D]],
                        )
                        nc.sync.dma_start(out=dst, in_=hp[:])
                    else:
                        nc.vector.tensor_tensor(
                            out=hstage[:, k * BPC:(k + 1) * BPC, :], in0=psH[:],
                            in1=dv[:, k * BPC:(k + 1) * BPC, None]
                            .to_broadcast([128, BPC, D]),
                            op=mybir.AluOpType.mult,
                        )
                if "noag" in debug:
                    return
                if not full:
                    ab = ag_ins[layer][:, :]
                    dst = bass.AP(
                        ab.tensor, ab.offset, [[D, 128], [BLK * D, nblocks], [1, D]]
                    )
                    nc.sync.dma_start(out=dst, in_=hstage[:])
                    nc.gpsimd.collective_compute(
                        "AllGather", mybir.AluOpType.bypass,
                        replica_groups=[list(range(NC))],
                        ins=[ab],
                        outs=[bass.AP(tabt, tab_off, [[D, m.NT], [1, D]])],
                    )

            def c_phase(layer):
                tv = lambda r, p: table_view(layer, r, p)
                gsum_ps = ps_pool.tile([32, D], f32, tag="gsum")
                nc.vector.memset(gmax[:], 0.0)
                job_i = 0
                jobs = m.jobs
                for k in range(nchunks):
                    agg = ps_agg.tile([128, BPC, D], f32, tag="agg")
                    nc.tensor.matmul(
                        out=agg[:], lhsT=zb[:, 0:128], rhs=zb[:, 0:BPC * D],
                        start=True, stop=False,
                    )
                    gts = []
                    for g in range(NGRP):
                        S = int(m.Sgrp[k, g])
                        if S == 0 or "nogather" in debug or ("nojobs" not in debug and False):
                            gts.append(None)
                            continue
                        o = int(m.gi_off[k, g])
                        gt = gatp.tile([128, S // 128, 128], bf16, tag="gt")
                        nc.gpsimd.dma_gather(
                            out_ap=gt[:],
                            in_ap=tv(g // 2, g % 2),
                            idxs_ap=idx_t[:, o // 16:(o + S) // 16],
                            num_idxs=S, num_idxs_reg=S, elem_size=128,
                            single_packet=False,
                        )
                        gts.append(gt)
                    nj = 0
                    while job_i + nj < NJ and jobs[job_i + nj][0] == k:
                        nj += 1
                    if "nogather" in debug:
                        nj = 0
                        job_i = [j for j in range(NJ + 1) if j == NJ or jobs[j][0] > k][0] if False else job_i
                    if nj and "nogather" not in debug:
                        lt = lhsp.tile([128, 32 * nj], bf16, tag="lt")
                        nc.sync.dma_start(
                            out=lt[:], in_=lhs_ext[:, 32 * job_i:32 * (job_i + nj)]
                        )
                    for jj in range(nj if ("nogather" not in debug and "nojobs" not in debug) else 0):
                        _, g, t, jb, wb, st, sp = jobs[job_i + jj]
                        nc.tensor.matmul(
                            out=agg[wb:wb + 32, jb, :],
                            lhsT=lt[:, 32 * jj:32 * jj + 32],
                            rhs=gts[g][:, t, 0:D],
                            start=st, stop=sp,
                            tile_position=(0, wb),
                        )
                    job_i += nj
                    nc.tensor.matmul(
                        out=agg[:], lhsT=zb[:, 0:128], rhs=zb[:, 0:BPC * D],
                        start=False, stop=True,
                    )
                    for b in range(BPC):
                        blk = k * BPC + b
                        nc.scalar.activation(
                            out=x_cur[:, blk, :], in_=agg[:, b, :],
                            func=mybir.ActivationFunctionType.Relu,
                            scale=disb_t[:, blk:blk + 1],
                        )
                        nc.tensor.matmul(
                            out=gsum_ps[0:gpc, :], lhsT=mem_t[:, blk, :],
                            rhs=x_cur[:, blk, :], start=(blk == 0),
                            stop=(blk == nblocks - 1),
                        )
                        gs = blk // m.blocks_per_graph
                        nc.vector.tensor_tensor(
                            out=gmax[:, gs, :], in0=gmax[:, gs, :],
                            in1=x_cur[:, blk, :], op=mybir.AluOpType.max,
                        )
                nc.scalar.activation(
                    out=psum_sb[0:gpc, layer, :], in_=gsum_ps[0:gpc, :],
                    func=mybir.ActivationFunctionType.Copy,
                )
                for gs in range(gpc):
                    psT = ps_t.tile([D, 128], f32, tag="psT")
                    nc.tensor.transpose(
                        out=psT[:], in_=gmax[:, gs, :], identity=ident_t[:]
                    )
                    nc.vector.tensor_reduce(
                        out=pool_sb[:, layer, gs:gs + 1], in_=psT[:],
                        axis=mybir.AxisListType.X, op=mybir.AluOpType.max,
                    )

            for _ in range(repeat):
                a_phase(0, full=True)
                c_phase(0)
                a_phase(1, full=False)
                c_phase(1)
                a_phase(2, full=False)
                c_phase(2)

            xo = xout_ext[:, :]
            nc.sync.dma_start(
                out=bass.AP(xo.tensor, 0, [[D, 128], [BLK * D, nblocks], [1, D]]),
                in_=x_cur[:],
            )
            nc.sync.dma_start(out=pmax_ext[:], in_=pool_sb[:])
            nc.sync.dma_start(out=psm_ext[:], in_=psum_sb[:])

    nc.compile()
    return nc


def _in_maps(m):
    w = np.stack([W.astype(ml_dtypes.bfloat16) for W in m.Ws], axis=1)
    ident = np.eye(128, dtype=np.float32)
    return [
        {
            "xT": m.xT,
            "idx": m.idx_wrapped[c],
            "lhs": m.lhs[c],
            "disb": m.disb[c],
            "disall": m.disall,
            "member": m.member[c],
            "w": w,
            "ident": ident,
        }
        for c in range(NC)
    ]


def _assemble(m, results):
    xfull = np.zeros((m.NT, D), np.float32)
    for c in range(NC):
        xfull[c * m.core_nodes:(c + 1) * m.core_nodes] = results[c]["xout"]
    x_out = xfull[m.pid]

    gemb = np.zeros((m.G, 2 * D), np.float32)
    for c in range(NC):
        pmax = results[c]["pmax"]      # [D, L, gpc]
        psm = results[c]["psumout"]    # [32, L, D]
        for gs in range(m.g_per_core):
            g = c * m.g_per_core + gs
            for layer in range(L):
                gemb[g, :D] += pmax[:, layer, gs]
                gemb[g, D:] += psm[gs, layer, :] / m.counts[g]
    return x_out, gemb


def kernel(x, edge_index, batch, W1, b1, W2, b2, W3, b3):
    x = np.asarray(x, np.float32)
    edge_index = np.asarray(edge_index)
    batch = np.asarray(batch).astype(np.int64)
    m = _preprocess(x, edge_index, batch, [W1, W2, W3], [b1, b2, b3])
    nc = _build_kernel(m)
    res = run_bass_kernel_spmd(nc, _in_maps(m), list(range(NC)), trace=False)
    return _assemble(m, res.results)
